# revision 6
# baseline (speedup 1.0000x reference)
"""Trainium2 Bass kernel for nn_MultiHeadGraphAttention (v2).

Multi-head graph attention (GAT-style), one head per NeuronCore:
    h_prime = einsum('nf,hfo->hno', h, w)
    attn    = softmax(where(adj, leakyrelu(s_i + d_j), -inf), axis=-1)
    out     = attn @ h_prime + b

Key identity: softmax is invariant to any per-row scale, so with
x = s_i + d_j, u = exp(s), v = exp(d), v2 = exp(0.2 d), g = exp(-0.8 s):

    p[i,j] = m * exp(leakyrelu(x)) = u_i * q[j,i]
    q[j,i] = m * max(v_j, g_i * v2_j)          (u_i cancels in the softmax)

so out = (Vaug^T @ q) normalized by the ones-column, where only q needs
per-element work.  Per [128, 4096] chunk of q^T (j on partitions):

  - "ts" route:  ONE fused DVE tensor_scalar  max(G * v2_j, v_j)  (4x mode)
                 + ONE DVE tensor_tensor mask multiply (2x mode)
  - "act" route: q = exp(0.8*relu(-(x)) + d_j) via 2 ScalarE passes
                 (Relu scale=-0.8 bias=-0.8 d_j, then Exp bias=d_j)
                 + the same DVE mask multiply  (balances ACT vs DVE)
  - "gts" route: like "ts" but the fused tensor_scalar runs on GpSimd.

The adjacency mask lives in HBM as uint8 (16 MB instead of 32 MB) and is
cast u8 -> bf16 in flight by the SWDGE (gpsimd) DMA engines.

s/d rows are computed as [2, N] = wa2^T @ h^T with bf16 hi/lo split
compensation, broadcast across partitions via a small DRAM bounce, and
d is transposed into per-partition columns with tiny PE transposes.
"""
import sys

if "/opt/trn_rl_repo" not in sys.path:
    sys.path.insert(0, "/opt/trn_rl_repo")

from contextlib import ExitStack

import ml_dtypes
import numpy as np

import concourse.bass as bass
import concourse.bacc as bacc
import concourse.tile as tile
from concourse import mybir
from concourse.bass_utils import run_bass_kernel_spmd

F32 = mybir.dt.float32
BF16 = mybir.dt.bfloat16
U8 = mybir.dt.uint8
AF = mybir.ActivationFunctionType
ALU = mybir.AluOpType

N = 4096
F_IN = 256
N_HEAD = 8
F_OUT = 64
NEG = 0.2
NCH = N // 128        # 32 j-chunks
NSL = N // 512        # 8 512-slices
FC = F_IN // 128      # 2 f-chunks
VW = F_OUT + 1        # 65: V columns + ones column

# Per-chunk route: "ts" (fused DVE), "act" (2x ScalarE), "gts" (GpSimd).
_ACT = {1, 3, 5, 7, 9, 11, 13, 15, 17, 19, 21, 23}
_GTS = {25, 29}
ROUTES = ["act" if i in _ACT else ("gts" if i in _GTS else "ts")
          for i in range(NCH)]


def build_program(routes=ROUTES):
    nc = bacc.Bacc("TRN2", target_bir_lowering=False, debug=False)
    hT_hi = nc.dram_tensor("hT_hi", [F_IN, N], BF16, kind="ExternalInput").ap()
    hT_lo = nc.dram_tensor("hT_lo", [F_IN, N], BF16, kind="ExternalInput").ap()
    w_bf = nc.dram_tensor("w_bf", [F_IN, F_OUT], BF16, kind="ExternalInput").ap()
    wa2_hi = nc.dram_tensor("wa2_hi", [F_IN, 2], BF16, kind="ExternalInput").ap()
    wa2_lo = nc.dram_tensor("wa2_lo", [F_IN, 2], BF16, kind="ExternalInput").ap()
    b_row = nc.dram_tensor("b_row", [1, F_OUT], BF16, kind="ExternalInput").ap()
    mask_u8 = nc.dram_tensor("mask_u8", [N, N], U8, kind="ExternalInput").ap()
    eye2 = nc.dram_tensor("eye2", [2, 2], F32, kind="ExternalInput").ap()
    outT = nc.dram_tensor("outT", [F_OUT, N], F32, kind="ExternalOutput").ap()
    s_dram = nc.dram_tensor("s_scratch", [N], BF16).ap()
    sv_dram = nc.dram_tensor("sv_scratch", [N], F32).ap()
    r_dram = nc.dram_tensor("r_scratch", [N], F32).ap()

    with tile.TileContext(nc) as tc, ExitStack() as ctx:
        const_pool = ctx.enter_context(tc.tile_pool(name="const", bufs=1))
        pre_ctx = ExitStack()
        pre_pool = pre_ctx.enter_context(tc.tile_pool(name="pre", bufs=1))
        psw_pool = pre_ctx.enter_context(tc.tile_pool(name="psw", bufs=2, space="PSUM"))
        psd_pool = pre_ctx.enter_context(tc.tile_pool(name="psd", bufs=1, space="PSUM"))
        psv_pool = pre_ctx.enter_context(tc.tile_pool(name="psv", bufs=2, space="PSUM"))

        # ---------------- constants / small loads ----------------
        wa2hi_sb = const_pool.tile([128, FC * 2], BF16, tag="wa2hi")
        wa2lo_sb = const_pool.tile([128, FC * 2], BF16, tag="wa2lo")
        for fc in range(FC):
            nc.sync.dma_start(wa2hi_sb[:, fc * 2:(fc + 1) * 2],
                              wa2_hi[fc * 128:(fc + 1) * 128, :])
            nc.sync.dma_start(wa2lo_sb[:, fc * 2:(fc + 1) * 2],
                              wa2_lo[fc * 128:(fc + 1) * 128, :])
        w_sb = const_pool.tile([128, FC * F_OUT], BF16, tag="w")
        for fc in range(FC):
            nc.sync.dma_start(w_sb[:, fc * F_OUT:(fc + 1) * F_OUT],
                              w_bf[fc * 128:(fc + 1) * 128, :])
        brow_sb = const_pool.tile([1, F_OUT], BF16, tag="brow")
        nc.sync.dma_start(brow_sb[:, :], b_row[:, :])
        ones_row = const_pool.tile([1, 128], BF16, tag="ones_row")
        nc.vector.memset(ones_row[:, :], 1.0)
        eye2_sb = const_pool.tile([2, 2], F32, tag="eye2")
        nc.sync.dma_start(eye2_sb[:, :], eye2[:, :])

        # warmup tile for PE clock (HAM gate releases after ~4us busy)
        wu_t = pre_pool.tile([128, 512], BF16, tag="wu")
        nc.vector.memset(wu_t[:, :], 0.0)
        ps_wu = psw_pool.tile([128, 512], F32, tag="pswu")
        for _ in range(8):
            nc.tensor.matmul(ps_wu[:, :], wu_t[:, 0:128], wu_t[:, :],
                             start=True, stop=True)

        # ---------------- hT staging (slice-ordered for pipelining) ----------
        hThi_sb = pre_pool.tile([128, FC * N], BF16, tag="hThi")
        hTlo_sb = pre_pool.tile([128, FC * N], BF16, tag="hTlo")
        for sl in range(NSL):
            s0, s1 = sl * 512, (sl + 1) * 512
            for fc in range(FC):
                nc.sync.dma_start(hThi_sb[:, fc * N + s0: fc * N + s1],
                                  hT_hi[fc * 128:(fc + 1) * 128, s0:s1])
            for fc in range(FC):
                nc.sync.dma_start(hTlo_sb[:, fc * N + s0: fc * N + s1],
                                  hT_lo[fc * 128:(fc + 1) * 128, s0:s1])

        # ---------------- s/d rows, hi/lo compensated ----------------
        # sdT[0,:] = s row, sdT[1,:] = d row
        sdT = pre_pool.tile([2, N], F32, tag="sdT")
        s_bf_row = pre_pool.tile([1, N], BF16, tag="sbfrow")
        combos = [(wa, hsb) for fc in range(FC)
                  for (wa, hsb) in ((wa2hi_sb, hThi_sb), (wa2hi_sb, hTlo_sb),
                                    (wa2lo_sb, hThi_sb))]
        for sl in range(NSL):
            s0, s1 = sl * 512, (sl + 1) * 512
            ps_sd = psw_pool.tile([2, 512], F32, tag="pssd")
            for ci, (wa, hsb) in enumerate(combos):
                fc = ci // 3
                nc.tensor.matmul(ps_sd[:, :], wa[:, fc * 2:(fc + 1) * 2],
                                 hsb[:, fc * N + s0: fc * N + s1],
                                 start=(ci == 0), stop=(ci == len(combos) - 1))
            nc.scalar.copy(sdT[0:2, s0:s1], ps_sd[:, :])
            nc.vector.tensor_copy(s_bf_row[0:1, s0:s1], sdT[0:1, s0:s1])
            nc.scalar.dma_start(s_dram[s0:s1], s_bf_row[0:1, s0:s1])

        # ---------------- d columns via PE transposes ----------------
        ps_dall = psd_pool.tile([128, 2 * NCH], F32, tag="psdall")
        for jc in range(NCH):
            nc.tensor.transpose(ps_dall[:, 2 * jc:2 * jc + 2],
                                sdT[0:2, jc * 128:(jc + 1) * 128], eye2_sb[:, :])
        sd_cols = const_pool.tile([128, 2 * NCH], F32, tag="sdcols")
        nc.vector.tensor_copy(sd_cols[:, :], ps_dall[:, :])
        # d_j per-partition columns live at sd_cols[:, 2*jc+1]
        dT_sb = const_pool.tile([128, NCH], F32, tag="dT")
        nc.vector.tensor_copy(dT_sb[:, :], sd_cols[:, 1::2])
        negd08 = const_pool.tile([128, NCH], F32, tag="negd08")
        nc.vector.tensor_scalar(negd08[:, :], dT_sb[:, :], -0.8, None, op0=ALU.mult)
        v_sb = const_pool.tile([128, NCH], F32, tag="v")
        nc.scalar.activation(v_sb[:, :], dT_sb[:, :], AF.Exp)
        v2_sb = const_pool.tile([128, NCH], F32, tag="v2")
        nc.scalar.activation(v2_sb[:, :], dT_sb[:, :], AF.Exp, scale=NEG)

        # ---------------- S / G broadcast tables ----------------
        S_b = const_pool.tile([128, N], BF16, tag="Sb")
        H2 = N // 2
        for half in range(2):
            hs = slice(half * H2, (half + 1) * H2)
            nc.sync.dma_start(S_b[:, hs], s_dram[None, hs].broadcast_to((128, H2)))
        G_b = const_pool.tile([128, N], BF16, tag="Gb")
        for half in range(2):
            hs = slice(half * H2, (half + 1) * H2)
            nc.scalar.activation(G_b[:, hs], S_b[:, hs], AF.Exp, scale=-0.8)

        # ---------------- h_prime (V, bf16), bias folded via ones row -------
        V_sb = const_pool.tile([128, NCH * VW], BF16, tag="V")
        nc.vector.memset(V_sb[:, :], 1.0)
        for jc in range(NCH):
            ps_v = psv_pool.tile([128, F_OUT], F32, tag="psv")
            for fc in range(FC):
                nc.tensor.matmul(
                    ps_v[:, :],
                    hThi_sb[:, fc * N + jc * 128: fc * N + (jc + 1) * 128],
                    w_sb[:, fc * F_OUT:(fc + 1) * F_OUT],
                    start=(fc == 0), stop=False)
            nc.tensor.matmul(ps_v[:, :], ones_row[:, :], brow_sb[:, :],
                             start=False, stop=True)
            nc.scalar.copy(V_sb[:, jc * VW: jc * VW + F_OUT], ps_v[:, :])

        # ---------------- attention j-loop ----------------
        pre_ctx.close()  # release hT staging + preamble PSUM
        mask_pool = ctx.enter_context(tc.tile_pool(name="maskp", bufs=4))
        q_pool = ctx.enter_context(tc.tile_pool(name="qp", bufs=3))
        r32_pool = ctx.enter_context(tc.tile_pool(name="r32p", bufs=2))
        p_pool = ctx.enter_context(tc.tile_pool(name="pp", bufs=3))
        psbig_pool = ctx.enter_context(tc.tile_pool(name="psbig", bufs=1, space="PSUM"))
        ps_A = psbig_pool.tile([VW, N], F32, tag="psA")
        for jc in range(NCH):
            m_t = mask_pool.tile([128, N], BF16, tag="mt")
            nc.gpsimd.dma_start(m_t[:, :], mask_u8[jc * 128:(jc + 1) * 128, :])
            q_t = q_pool.tile([128, N], BF16, tag="qt")
            if routes[jc] == "ts":
                nc.vector.tensor_scalar(q_t[:, :], G_b[:, :],
                                        v2_sb[:, jc:jc + 1], v_sb[:, jc:jc + 1],
                                        op0=ALU.mult, op1=ALU.max)
            elif routes[jc] == "gts":
                nc.gpsimd.tensor_scalar(q_t[:, :], G_b[:, :],
                                        v2_sb[:, jc:jc + 1], v_sb[:, jc:jc + 1],
                                        op0=ALU.mult, op1=ALU.max)
            else:
                r_t = r32_pool.tile([128, N], F32, tag="rt")
                nc.scalar.activation(r_t[:, :], S_b[:, :], AF.Relu,
                                     scale=-0.8, bias=negd08[:, jc:jc + 1])
                nc.scalar.activation(q_t[:, :], r_t[:, :], AF.Exp,
                                     bias=dT_sb[:, jc:jc + 1])
            p_t = p_pool.tile([128, N], BF16, tag="pt")
            nc.vector.tensor_tensor(p_t[:, :], q_t[:, :], m_t[:, :], op=ALU.mult)
            for k in range(NSL):
                nc.tensor.matmul(ps_A[:, k * 512:(k + 1) * 512],
                                 V_sb[:, jc * VW:(jc + 1) * VW],
                                 p_t[:, k * 512:(k + 1) * 512],
                                 start=(jc == 0), stop=(jc == NCH - 1))

        # ---------------- tail: normalize ----------------
        tail_pool = ctx.enter_context(tc.tile_pool(name="tail", bufs=1))
        Srow = tail_pool.tile([1, N], F32, tag="Srow")
        nc.scalar.copy(Srow[:, :], ps_A[F_OUT:VW, :])
        nc.scalar.dma_start(sv_dram[:], Srow[:, :])
        sres = tail_pool.tile([128, N // 128], F32, tag="sres")
        nc.scalar.dma_start(sres[:, :], sv_dram.rearrange("(p q) -> p q", p=128))
        rres = tail_pool.tile([128, N // 128], F32, tag="rres")
        nc.vector.reciprocal(rres[:, :], sres[:, :])
        nc.scalar.dma_start(r_dram.rearrange("(p q) -> p q", p=128), rres[:, :])
        R_sb = tail_pool.tile([F_OUT, N], F32, tag="Rsb")
        F_sb = tail_pool.tile([F_OUT, N], F32, tag="Fsb")
        for half in range(2):
            hs = slice(half * H2, (half + 1) * H2)
            nc.scalar.dma_start(R_sb[:, hs],
                                r_dram[None, hs].broadcast_to((F_OUT, H2)))
            nc.vector.tensor_tensor(F_sb[:, hs], ps_A[0:F_OUT, hs], R_sb[:, hs],
                                    op=ALU.mult)
            nc.scalar.dma_start(outT[:, hs], F_sb[:, hs])
    nc.compile()
    return nc


_CACHED_NC = None


def _get_nc():
    global _CACHED_NC
    if _CACHED_NC is None:
        _CACHED_NC = build_program()
    return _CACHED_NC


def _split_hilo(x):
    hi = x.astype(ml_dtypes.bfloat16)
    lo = (x - hi.astype(np.float32)).astype(ml_dtypes.bfloat16)
    return hi, lo


def _prep_inputs(h, adj, w, a_src, a_dst, b):
    h = np.asarray(h, dtype=np.float32)
    adj = np.asarray(adj)
    w = np.asarray(w, dtype=np.float32)
    a_src = np.asarray(a_src, dtype=np.float32)
    a_dst = np.asarray(a_dst, dtype=np.float32)
    b = np.asarray(b, dtype=np.float32)

    h_T = np.ascontiguousarray(h.T)
    hT_hi, hT_lo = _split_hilo(h_T)
    mask_u8 = np.ascontiguousarray(adj.T).astype(np.uint8)
    b_row = np.ascontiguousarray(b[None, :]).astype(ml_dtypes.bfloat16)

    in_maps = []
    for c in range(N_HEAD):
        wa_src = (w[c] @ a_src[c])[:, 0]              # [F_IN]
        wa_dst = (w[c] @ a_dst[c])[:, 0]
        cols = np.stack([wa_src, wa_dst], axis=1)     # [F_IN, 2]
        wa_hi, wa_lo = _split_hilo(cols)
        in_maps.append({
            "hT_hi": hT_hi,
            "hT_lo": hT_lo,
            "w_bf": np.ascontiguousarray(w[c]).astype(ml_dtypes.bfloat16),
            "wa2_hi": np.ascontiguousarray(wa_hi),
            "wa2_lo": np.ascontiguousarray(wa_lo),
            "b_row": b_row,
            "mask_u8": mask_u8,
            "eye2": np.eye(2, dtype=np.float32),
        })
    return in_maps


def _run(in_maps, trace=False, **kwargs):
    nc = _get_nc()
    return run_bass_kernel_spmd(nc, in_maps, list(range(N_HEAD)), trace=trace, **kwargs)


def kernel(h, adj, w, a_src, a_dst, b):
    in_maps = _prep_inputs(h, adj, w, a_src, a_dst, b)
    res = _run(in_maps)
    out = np.stack([np.ascontiguousarray(res.results[c]["outT"].T)
                    for c in range(N_HEAD)])
    return out.astype(np.float32)


# revision 7
# speedup vs baseline: 1.4387x; 1.4387x over previous
"""Trainium2 Bass kernel for nn_MultiHeadGraphAttention (v2).

Multi-head graph attention (GAT-style), one head per NeuronCore:
    h_prime = einsum('nf,hfo->hno', h, w)
    attn    = softmax(where(adj, leakyrelu(s_i + d_j), -inf), axis=-1)
    out     = attn @ h_prime + b

Key identity: softmax is invariant to any per-row scale, so with
x = s_i + d_j, u = exp(s), v = exp(d), v2 = exp(0.2 d), g = exp(-0.8 s):

    p[i,j] = m * exp(leakyrelu(x)) = u_i * q[j,i]
    q[j,i] = m * max(v_j, g_i * v2_j)          (u_i cancels in the softmax)

so out = (Vaug^T @ q) normalized by the ones-column, where only q needs
per-element work.  Per [128, 4096] chunk of q^T (j on partitions):

  - "ts" route:  ONE fused DVE tensor_scalar  max(G * v2_j, v_j)  (4x mode)
                 + ONE DVE tensor_tensor mask multiply (2x mode)
  - "act" route: q = exp(0.8*relu(-(x)) + d_j) via 2 ScalarE passes
                 (Relu scale=-0.8 bias=-0.8 d_j, then Exp bias=d_j)
                 + the same DVE mask multiply  (balances ACT vs DVE)
  - "gts" route: like "ts" but the fused tensor_scalar runs on GpSimd.

The adjacency mask lives in HBM as uint8 (16 MB instead of 32 MB) and is
cast u8 -> bf16 in flight by the SWDGE (gpsimd) DMA engines.

s/d rows are computed as [2, N] = wa2^T @ h^T with bf16 hi/lo split
compensation, broadcast across partitions via a small DRAM bounce, and
d is transposed into per-partition columns with tiny PE transposes.
"""
import sys

if "/opt/trn_rl_repo" not in sys.path:
    sys.path.insert(0, "/opt/trn_rl_repo")

from contextlib import ExitStack

import ml_dtypes
import numpy as np

import concourse.bass as bass
import concourse.bacc as bacc
import concourse.tile as tile
from concourse import mybir
from concourse.bass_utils import run_bass_kernel_spmd

F32 = mybir.dt.float32
BF16 = mybir.dt.bfloat16
U8 = mybir.dt.uint8
AF = mybir.ActivationFunctionType
ALU = mybir.AluOpType

N = 4096
F_IN = 256
N_HEAD = 8
F_OUT = 64
NEG = 0.2
NCH = N // 128        # 32 j-chunks
NSL = N // 512        # 8 512-slices
FC = F_IN // 128      # 2 f-chunks
VW = F_OUT + 1        # 65: V columns + ones column

# Per-chunk route: "ts" (fused DVE) or "act" (2x ScalarE).
# NOTE: gpsimd.tensor_scalar measured 60us/op AND stalls concurrent DVE
# 2-port ops via the shared SBUF port -- never use GpSimd for elementwise.
_ACT = {1, 3, 5, 8, 10, 13, 15, 18, 20, 23, 25, 28}
ROUTES = ["act" if i in _ACT else "ts" for i in range(NCH)]


def build_program(routes=ROUTES):
    nc = bacc.Bacc("TRN2", target_bir_lowering=False, debug=False)
    hT_hi = nc.dram_tensor("hT_hi", [F_IN, N], BF16, kind="ExternalInput").ap()
    hT_lo = nc.dram_tensor("hT_lo", [F_IN, N], BF16, kind="ExternalInput").ap()
    w_bf = nc.dram_tensor("w_bf", [F_IN, F_OUT], BF16, kind="ExternalInput").ap()
    wa2_hi = nc.dram_tensor("wa2_hi", [F_IN, 2], BF16, kind="ExternalInput").ap()
    wa2_lo = nc.dram_tensor("wa2_lo", [F_IN, 2], BF16, kind="ExternalInput").ap()
    b_row = nc.dram_tensor("b_row", [1, F_OUT], BF16, kind="ExternalInput").ap()
    mask_u8 = nc.dram_tensor("mask_u8", [N, N], U8, kind="ExternalInput").ap()
    eye2 = nc.dram_tensor("eye2", [2, 2], F32, kind="ExternalInput").ap()
    outT = nc.dram_tensor("outT", [F_OUT, N], F32, kind="ExternalOutput").ap()
    s_dram = nc.dram_tensor("s_scratch", [N], BF16).ap()
    sv_dram = nc.dram_tensor("sv_scratch", [N], F32).ap()
    r_dram = nc.dram_tensor("r_scratch", [N], F32).ap()

    with tile.TileContext(nc) as tc, ExitStack() as ctx:
        const_pool = ctx.enter_context(tc.tile_pool(name="const", bufs=1))
        pre_ctx = ExitStack()
        pre_pool = pre_ctx.enter_context(tc.tile_pool(name="pre", bufs=1))
        psw_pool = pre_ctx.enter_context(tc.tile_pool(name="psw", bufs=2, space="PSUM"))
        psd_pool = pre_ctx.enter_context(tc.tile_pool(name="psd", bufs=1, space="PSUM"))
        psv_pool = pre_ctx.enter_context(tc.tile_pool(name="psv", bufs=2, space="PSUM"))

        # ---------------- constants / small loads ----------------
        wa2hi_sb = const_pool.tile([128, FC * 2], BF16, tag="wa2hi")
        wa2lo_sb = const_pool.tile([128, FC * 2], BF16, tag="wa2lo")
        for fc in range(FC):
            nc.sync.dma_start(wa2hi_sb[:, fc * 2:(fc + 1) * 2],
                              wa2_hi[fc * 128:(fc + 1) * 128, :])
            nc.sync.dma_start(wa2lo_sb[:, fc * 2:(fc + 1) * 2],
                              wa2_lo[fc * 128:(fc + 1) * 128, :])
        w_sb = const_pool.tile([128, FC * F_OUT], BF16, tag="w")
        for fc in range(FC):
            nc.sync.dma_start(w_sb[:, fc * F_OUT:(fc + 1) * F_OUT],
                              w_bf[fc * 128:(fc + 1) * 128, :])
        brow_sb = const_pool.tile([1, F_OUT], BF16, tag="brow")
        nc.sync.dma_start(brow_sb[:, :], b_row[:, :])
        ones_row = const_pool.tile([1, 128], BF16, tag="ones_row")
        nc.vector.memset(ones_row[:, :], 1.0)
        eye2_sb = const_pool.tile([2, 2], F32, tag="eye2")
        nc.sync.dma_start(eye2_sb[:, :], eye2[:, :])

        # warmup tile for PE clock (HAM gate releases after ~4us busy)
        wu_t = pre_pool.tile([128, 512], BF16, tag="wu")
        nc.vector.memset(wu_t[:, :], 0.0)
        ps_wu = psw_pool.tile([128, 512], F32, tag="pswu")
        for _ in range(8):
            nc.tensor.matmul(ps_wu[:, :], wu_t[:, 0:128], wu_t[:, :],
                             start=True, stop=True)

        # ---------------- hT staging (slice-ordered for pipelining) ----------
        hThi_sb = pre_pool.tile([128, FC * N], BF16, tag="hThi")
        hTlo_sb = pre_pool.tile([128, FC * N], BF16, tag="hTlo")
        for sl in range(NSL):
            s0, s1 = sl * 512, (sl + 1) * 512
            for fc in range(FC):
                nc.sync.dma_start(hThi_sb[:, fc * N + s0: fc * N + s1],
                                  hT_hi[fc * 128:(fc + 1) * 128, s0:s1])
            for fc in range(FC):
                nc.sync.dma_start(hTlo_sb[:, fc * N + s0: fc * N + s1],
                                  hT_lo[fc * 128:(fc + 1) * 128, s0:s1])

        # ---------------- s/d rows, hi/lo compensated ----------------
        # sdT[0,:] = s row, sdT[1,:] = d row
        sdT = pre_pool.tile([2, N], F32, tag="sdT")
        s_bf_row = pre_pool.tile([1, N], BF16, tag="sbfrow")
        combos = [(wa, hsb) for fc in range(FC)
                  for (wa, hsb) in ((wa2hi_sb, hThi_sb), (wa2hi_sb, hTlo_sb),
                                    (wa2lo_sb, hThi_sb))]
        for sl in range(NSL):
            s0, s1 = sl * 512, (sl + 1) * 512
            ps_sd = psw_pool.tile([2, 512], F32, tag="pssd")
            for ci, (wa, hsb) in enumerate(combos):
                fc = ci // 3
                nc.tensor.matmul(ps_sd[:, :], wa[:, fc * 2:(fc + 1) * 2],
                                 hsb[:, fc * N + s0: fc * N + s1],
                                 start=(ci == 0), stop=(ci == len(combos) - 1))
            nc.scalar.copy(sdT[0:2, s0:s1], ps_sd[:, :])
            nc.vector.tensor_copy(s_bf_row[0:1, s0:s1], sdT[0:1, s0:s1])
            nc.scalar.dma_start(s_dram[s0:s1], s_bf_row[0:1, s0:s1])

        # ---------------- d columns via PE transposes ----------------
        ps_dall = psd_pool.tile([128, 2 * NCH], F32, tag="psdall")
        for jc in range(NCH):
            nc.tensor.transpose(ps_dall[:, 2 * jc:2 * jc + 2],
                                sdT[0:2, jc * 128:(jc + 1) * 128], eye2_sb[:, :])
        sd_cols = const_pool.tile([128, 2 * NCH], F32, tag="sdcols")
        nc.vector.tensor_copy(sd_cols[:, :], ps_dall[:, :])
        # d_j per-partition columns live at sd_cols[:, 2*jc+1]
        dT_sb = const_pool.tile([128, NCH], F32, tag="dT")
        nc.vector.tensor_copy(dT_sb[:, :], sd_cols[:, 1::2])
        negd08 = const_pool.tile([128, NCH], F32, tag="negd08")
        nc.vector.tensor_scalar(negd08[:, :], dT_sb[:, :], -0.8, None, op0=ALU.mult)
        v_sb = const_pool.tile([128, NCH], F32, tag="v")
        nc.scalar.activation(v_sb[:, :], dT_sb[:, :], AF.Exp)
        v2_sb = const_pool.tile([128, NCH], F32, tag="v2")
        nc.scalar.activation(v2_sb[:, :], dT_sb[:, :], AF.Exp, scale=NEG)

        # ---------------- S / G broadcast tables ----------------
        S_b = const_pool.tile([128, N], BF16, tag="Sb")
        H2 = N // 2
        for half in range(2):
            hs = slice(half * H2, (half + 1) * H2)
            nc.sync.dma_start(S_b[:, hs], s_dram[None, hs].broadcast_to((128, H2)))
        G_b = const_pool.tile([128, N], BF16, tag="Gb")
        for half in range(2):
            hs = slice(half * H2, (half + 1) * H2)
            nc.scalar.activation(G_b[:, hs], S_b[:, hs], AF.Exp, scale=-0.8)

        # ---------------- h_prime (V, bf16), bias folded via ones row -------
        V_sb = const_pool.tile([128, NCH * VW], BF16, tag="V")
        nc.vector.memset(V_sb[:, :], 1.0)
        for jc in range(NCH):
            ps_v = psv_pool.tile([128, F_OUT], F32, tag="psv")
            for fc in range(FC):
                nc.tensor.matmul(
                    ps_v[:, :],
                    hThi_sb[:, fc * N + jc * 128: fc * N + (jc + 1) * 128],
                    w_sb[:, fc * F_OUT:(fc + 1) * F_OUT],
                    start=(fc == 0), stop=False)
            nc.tensor.matmul(ps_v[:, :], ones_row[:, :], brow_sb[:, :],
                             start=False, stop=True)
            nc.scalar.copy(V_sb[:, jc * VW: jc * VW + F_OUT], ps_v[:, :])

        # ---------------- attention j-loop ----------------
        pre_ctx.close()  # release hT staging + preamble PSUM
        mask_pool = ctx.enter_context(tc.tile_pool(name="maskp", bufs=4))
        q_pool = ctx.enter_context(tc.tile_pool(name="qp", bufs=3))
        r32_pool = ctx.enter_context(tc.tile_pool(name="r32p", bufs=2))
        p_pool = ctx.enter_context(tc.tile_pool(name="pp", bufs=3))
        psbig_pool = ctx.enter_context(tc.tile_pool(name="psbig", bufs=1, space="PSUM"))
        ps_A = psbig_pool.tile([VW, N], F32, tag="psA")
        for jc in range(NCH):
            m_t = mask_pool.tile([128, N], BF16, tag="mt")
            nc.gpsimd.dma_start(m_t[:, :], mask_u8[jc * 128:(jc + 1) * 128, :])
            q_t = q_pool.tile([128, N], BF16, tag="qt")
            if routes[jc] == "ts":
                nc.vector.tensor_scalar(q_t[:, :], G_b[:, :],
                                        v2_sb[:, jc:jc + 1], v_sb[:, jc:jc + 1],
                                        op0=ALU.mult, op1=ALU.max)
            elif routes[jc] == "gts":
                nc.gpsimd.tensor_scalar(q_t[:, :], G_b[:, :],
                                        v2_sb[:, jc:jc + 1], v_sb[:, jc:jc + 1],
                                        op0=ALU.mult, op1=ALU.max)
            else:
                r_t = r32_pool.tile([128, N], F32, tag="rt")
                nc.scalar.activation(r_t[:, :], S_b[:, :], AF.Relu,
                                     scale=-0.8, bias=negd08[:, jc:jc + 1])
                nc.scalar.activation(q_t[:, :], r_t[:, :], AF.Exp,
                                     bias=dT_sb[:, jc:jc + 1])
            p_t = p_pool.tile([128, N], BF16, tag="pt")
            nc.vector.tensor_tensor(p_t[:, :], q_t[:, :], m_t[:, :], op=ALU.mult)
            for k in range(NSL):
                nc.tensor.matmul(ps_A[:, k * 512:(k + 1) * 512],
                                 V_sb[:, jc * VW:(jc + 1) * VW],
                                 p_t[:, k * 512:(k + 1) * 512],
                                 start=(jc == 0), stop=(jc == NCH - 1))

        # ---------------- tail: normalize ----------------
        tail_pool = ctx.enter_context(tc.tile_pool(name="tail", bufs=1))
        Srow = tail_pool.tile([1, N], F32, tag="Srow")
        nc.scalar.copy(Srow[:, :], ps_A[F_OUT:VW, :])
        nc.scalar.dma_start(sv_dram[:], Srow[:, :])
        sres = tail_pool.tile([128, N // 128], F32, tag="sres")
        nc.scalar.dma_start(sres[:, :], sv_dram.rearrange("(p q) -> p q", p=128))
        rres = tail_pool.tile([128, N // 128], F32, tag="rres")
        nc.vector.reciprocal(rres[:, :], sres[:, :])
        nc.scalar.dma_start(r_dram.rearrange("(p q) -> p q", p=128), rres[:, :])
        R_sb = tail_pool.tile([F_OUT, N], F32, tag="Rsb")
        F_sb = tail_pool.tile([F_OUT, N], F32, tag="Fsb")
        for half in range(2):
            hs = slice(half * H2, (half + 1) * H2)
            nc.scalar.dma_start(R_sb[:, hs],
                                r_dram[None, hs].broadcast_to((F_OUT, H2)))
            nc.vector.tensor_tensor(F_sb[:, hs], ps_A[0:F_OUT, hs], R_sb[:, hs],
                                    op=ALU.mult)
            nc.scalar.dma_start(outT[:, hs], F_sb[:, hs])
    nc.compile()
    return nc


_CACHED_NC = None


def _get_nc():
    global _CACHED_NC
    if _CACHED_NC is None:
        _CACHED_NC = build_program()
    return _CACHED_NC


def _split_hilo(x):
    hi = x.astype(ml_dtypes.bfloat16)
    lo = (x - hi.astype(np.float32)).astype(ml_dtypes.bfloat16)
    return hi, lo


def _prep_inputs(h, adj, w, a_src, a_dst, b):
    h = np.asarray(h, dtype=np.float32)
    adj = np.asarray(adj)
    w = np.asarray(w, dtype=np.float32)
    a_src = np.asarray(a_src, dtype=np.float32)
    a_dst = np.asarray(a_dst, dtype=np.float32)
    b = np.asarray(b, dtype=np.float32)

    h_T = np.ascontiguousarray(h.T)
    hT_hi, hT_lo = _split_hilo(h_T)
    mask_u8 = np.ascontiguousarray(adj.T).astype(np.uint8)
    b_row = np.ascontiguousarray(b[None, :]).astype(ml_dtypes.bfloat16)

    in_maps = []
    for c in range(N_HEAD):
        wa_src = (w[c] @ a_src[c])[:, 0]              # [F_IN]
        wa_dst = (w[c] @ a_dst[c])[:, 0]
        cols = np.stack([wa_src, wa_dst], axis=1)     # [F_IN, 2]
        wa_hi, wa_lo = _split_hilo(cols)
        in_maps.append({
            "hT_hi": hT_hi,
            "hT_lo": hT_lo,
            "w_bf": np.ascontiguousarray(w[c]).astype(ml_dtypes.bfloat16),
            "wa2_hi": np.ascontiguousarray(wa_hi),
            "wa2_lo": np.ascontiguousarray(wa_lo),
            "b_row": b_row,
            "mask_u8": mask_u8,
            "eye2": np.eye(2, dtype=np.float32),
        })
    return in_maps


def _run(in_maps, trace=False, **kwargs):
    nc = _get_nc()
    return run_bass_kernel_spmd(nc, in_maps, list(range(N_HEAD)), trace=trace, **kwargs)


def kernel(h, adj, w, a_src, a_dst, b):
    in_maps = _prep_inputs(h, adj, w, a_src, a_dst, b)
    res = _run(in_maps)
    out = np.stack([np.ascontiguousarray(res.results[c]["outT"].T)
                    for c in range(N_HEAD)])
    return out.astype(np.float32)


# revision 12
# speedup vs baseline: 1.4487x; 1.0070x over previous
"""Trainium2 Bass kernel for nn_MultiHeadGraphAttention (v4).

Multi-head graph attention (GAT-style), one head per NeuronCore:
    h_prime = einsum('nf,hfo->hno', h, w)
    attn    = softmax(where(adj, leakyrelu(s_i + d_j), -inf), axis=-1)
    out     = attn @ h_prime + b

Key identity: softmax is invariant to any per-column scale, so with
x = s_i + d_j, v = exp(d), v2 = exp(0.2 d), g = exp(-0.8 s):

    p[i,j] = m * exp(leakyrelu(x)) = exp(s_i) * q[j,i]
    q[j,i] = m * max(v_j, g_i * v2_j)       (exp(s_i) cancels in softmax)

out = (Vaug^T @ q) / ones-column.  Per-element work per [128, 4096] chunk
of q^T (j on partitions):

  - "ts" route:  ONE fused DVE tensor_scalar  max(G * v2_j, v_j)  (~4x)
  - "act" route: q = exp(0.8*relu(-(x)) + d_j) via 2 ScalarE passes
                 (Relu scale=-0.8 bias=-0.8 d_j, then Exp bias=d_j)
  plus one DVE tensor_tensor mask multiply, done on PAIRS of chunks
  ([128, 8192]) to amortize instruction overhead.  The mask is stored
  host-side in pair-layout bf16 so each pair is a single HWDGE DMA
  (SWDGE cast-DMA was measured to tax concurrent DVE 2-port ops ~20%
  via the shared descriptor-ring SBUF port, so HWDGE + bf16 wins).

Preamble: s/d rows via [2, N] = wa2^T @ h^T with bf16 hi/lo split
compensation (big 1MB hT DMAs split across both HWDGE rings), s
broadcast across partitions via a sliced DRAM bounce, d transposed into
per-partition columns with tiny PE transposes, h_prime bias folded into
the matmul via a ones row.  PE warmed up with dummy matmuls during the
hT loads.
"""
import sys

if "/opt/trn_rl_repo" not in sys.path:
    sys.path.insert(0, "/opt/trn_rl_repo")

from contextlib import ExitStack

import ml_dtypes
import numpy as np

import concourse.bass as bass
import concourse.bacc as bacc
import concourse.tile as tile
from concourse import mybir
from concourse.bass_utils import run_bass_kernel_spmd

F32 = mybir.dt.float32
BF16 = mybir.dt.bfloat16
U8 = mybir.dt.uint8
AF = mybir.ActivationFunctionType
ALU = mybir.AluOpType

N = 4096
F_IN = 256
N_HEAD = 8
F_OUT = 64
NEG = 0.2
NCH = N // 128        # 32 j-chunks
NPR = NCH // 2        # 16 chunk-pairs
NSL = N // 512        # 8 512-slices
FC = F_IN // 128      # 2 f-chunks
VW = F_OUT + 1        # 65: V columns + ones column

# const blob layout (bf16, [128, BLOB_W]):
#   [0:4)    wa2_hi (fc-major: fc0 s,d | fc1 s,d)
#   [4:8)    wa2_lo
#   [8:136)  w       (fc0 64 | fc1 64)
#   [136:200) b row  (row 0 only)
BLOB_W = 200

# Per-chunk route: "ts" (fused DVE tensor_scalar) or "act" (2x ScalarE).
_ACT = {4, 5, 10, 11, 16, 17, 22, 23, 28, 29}
ROUTES = ["act" if i in _ACT else "ts" for i in range(NCH)]


def build_program(routes=ROUTES):
    nc = bacc.Bacc("TRN2", target_bir_lowering=False, debug=False)
    hT_hi = nc.dram_tensor("hT_hi", [F_IN, N], BF16, kind="ExternalInput").ap()
    hT_lo = nc.dram_tensor("hT_lo", [F_IN, N], BF16, kind="ExternalInput").ap()
    blob = nc.dram_tensor("blob", [128, BLOB_W], BF16, kind="ExternalInput").ap()
    maskp = nc.dram_tensor("maskp", [NPR, 128, 2 * N], BF16,
                           kind="ExternalInput").ap()
    eye2 = nc.dram_tensor("eye2", [2, 2], F32, kind="ExternalInput").ap()
    outT = nc.dram_tensor("outT", [F_OUT, N], F32, kind="ExternalOutput").ap()
    s_dram = nc.dram_tensor("s_scratch", [N], BF16).ap()
    sv_dram = nc.dram_tensor("sv_scratch", [N], F32).ap()
    r_dram = nc.dram_tensor("r_scratch", [N], F32).ap()

    with tile.TileContext(nc) as tc, ExitStack() as ctx:
        const_pool = ctx.enter_context(tc.tile_pool(name="const", bufs=1))
        pre_ctx = ExitStack()
        pre_pool = pre_ctx.enter_context(tc.tile_pool(name="pre", bufs=1))
        pswu_pool = pre_ctx.enter_context(tc.tile_pool(name="pswu", bufs=1, space="PSUM"))
        psw_pool = pre_ctx.enter_context(tc.tile_pool(name="psw", bufs=2, space="PSUM"))
        psd_pool = pre_ctx.enter_context(tc.tile_pool(name="psd", bufs=1, space="PSUM"))
        psv_pool = pre_ctx.enter_context(tc.tile_pool(name="psv", bufs=2, space="PSUM"))

        # ---- warmup matmuls to release the PE HAM clock gate (no data dep)
        wu_t = pre_pool.tile([128, 512], BF16, tag="wu")
        nc.vector.memset(wu_t[:, :], 0.0)
        ps_wu = pswu_pool.tile([128, 512], F32, tag="pswu")
        for _ in range(10):
            nc.tensor.matmul(ps_wu[:, :], wu_t[:, 0:128], wu_t[:, :],
                             start=True, stop=True)

        # ---- big input loads: hi on sync ring, lo + blob on scalar ring
        hThi_sb = pre_pool.tile([128, FC * N], BF16, tag="hThi")
        hTlo_sb = pre_pool.tile([128, FC * N], BF16, tag="hTlo")
        for fc in range(FC):
            nc.sync.dma_start(hThi_sb[:, fc * N:(fc + 1) * N],
                              hT_hi[fc * 128:(fc + 1) * 128, :])
        blob_sb = const_pool.tile([128, BLOB_W], BF16, tag="blob")
        nc.scalar.dma_start(blob_sb[:, :], blob[:, :])
        for fc in range(FC):
            nc.scalar.dma_start(hTlo_sb[:, fc * N:(fc + 1) * N],
                                hT_lo[fc * 128:(fc + 1) * 128, :])
        wa2hi = blob_sb[:, 0:4]
        wa2lo = blob_sb[:, 4:8]
        w_sb = blob_sb[:, 8:136]
        brow_sb = blob_sb[0:1, 136:136 + F_OUT]
        ones_row = const_pool.tile([1, 128], BF16, tag="ones_row")
        nc.vector.memset(ones_row[:, :], 1.0)
        eye2_sb = const_pool.tile([2, 2], F32, tag="eye2")
        nc.scalar.dma_start(eye2_sb[:, :], eye2[:, :])

        # ---- s/d rows (hi/lo compensated), sliced; s -> bf16 row + DRAM
        s_bf_row = pre_pool.tile([1, N], BF16, tag="sbfrow")
        sdT = pre_pool.tile([2, N], F32, tag="sdT")
        combos = [(wa2hi, hThi_sb), (wa2hi, hTlo_sb), (wa2lo, hThi_sb)]
        for sl in range(NSL):
            s0, s1 = sl * 512, (sl + 1) * 512
            ps_sd = psw_pool.tile([2, 512], F32, tag="pssd")
            ci = 0
            for fc in range(FC):
                for (wa, hsb) in combos:
                    nc.tensor.matmul(ps_sd[:, :], wa[:, fc * 2:(fc + 1) * 2],
                                     hsb[:, fc * N + s0: fc * N + s1],
                                     start=(ci == 0), stop=(ci == 5))
                    ci += 1
            nc.scalar.copy(sdT[0:2, s0:s1], ps_sd[:, :])
            nc.vector.tensor_copy(s_bf_row[0:1, s0:s1], sdT[0:1, s0:s1])
            nc.scalar.dma_start(s_dram[s0:s1], s_bf_row[0:1, s0:s1])

        # ---- d columns via PE transposes -> tables
        ps_dall = psd_pool.tile([128, 2 * NCH], F32, tag="psdall")
        for jc in range(NCH):
            nc.tensor.transpose(ps_dall[:, 2 * jc:2 * jc + 2],
                                sdT[0:2, jc * 128:(jc + 1) * 128], eye2_sb[:, :])
        dT_sb = const_pool.tile([128, NCH], F32, tag="dT")
        nc.vector.tensor_copy(dT_sb[:, :], ps_dall[:, 1::2])
        negd08 = const_pool.tile([128, NCH], F32, tag="negd08")
        nc.vector.tensor_scalar(negd08[:, :], dT_sb[:, :], -0.8, None, op0=ALU.mult)
        v_sb = const_pool.tile([128, NCH], F32, tag="v")
        nc.scalar.activation(v_sb[:, :], dT_sb[:, :], AF.Exp)
        v2_sb = const_pool.tile([128, NCH], F32, tag="v2")
        nc.scalar.activation(v2_sb[:, :], dT_sb[:, :], AF.Exp, scale=NEG)

        # ---- S / G broadcast tables (single broadcast read from DRAM)
        S_b = const_pool.tile([128, N], BF16, tag="Sb")
        nc.sync.dma_start(S_b[:, :], s_dram[None, :].broadcast_to((128, N)))
        G_b = const_pool.tile([128, N], BF16, tag="Gb")
        H2 = N // 2
        for half in range(2):
            hs = slice(half * H2, (half + 1) * H2)
            nc.scalar.activation(G_b[:, hs], S_b[:, hs], AF.Exp, scale=-0.8)

        # ---- h_prime (V, bf16), bias folded in via ones row
        V_sb = const_pool.tile([128, NCH * VW], BF16, tag="V")
        nc.vector.memset(V_sb[:, :], 1.0)
        for jc in range(NCH):
            ps_v = psv_pool.tile([128, F_OUT], F32, tag="psv")
            for fc in range(FC):
                nc.tensor.matmul(
                    ps_v[:, :],
                    hThi_sb[:, fc * N + jc * 128: fc * N + (jc + 1) * 128],
                    w_sb[:, fc * F_OUT:(fc + 1) * F_OUT],
                    start=(fc == 0), stop=False)
            nc.tensor.matmul(ps_v[:, :], ones_row[:, :], brow_sb[:, :],
                             start=False, stop=True)
            nc.scalar.copy(V_sb[:, jc * VW: jc * VW + F_OUT], ps_v[:, :])

        # ---------------- attention j-loop over chunk PAIRS ----------------
        pre_ctx.close()
        mask_pool = ctx.enter_context(tc.tile_pool(name="maskp", bufs=2))
        q_pool = ctx.enter_context(tc.tile_pool(name="qp", bufs=2))
        r32_pool = ctx.enter_context(tc.tile_pool(name="r32p", bufs=2))
        p_pool = ctx.enter_context(tc.tile_pool(name="pp", bufs=2))
        psbig_pool = ctx.enter_context(tc.tile_pool(name="psbig", bufs=1, space="PSUM"))
        ps_A = psbig_pool.tile([VW, N], F32, tag="psA")
        for pc in range(NPR):
            m_t = mask_pool.tile([128, 2 * N], BF16, tag="mt")
            if pc % 2 == 0:
                nc.sync.dma_start(m_t[:, :], maskp[pc, :, :])
            else:
                nc.scalar.dma_start(m_t[:, :], maskp[pc, :, :])
            q_t = q_pool.tile([128, 2 * N], BF16, tag="qt")
            for half in (0, 1):
                jc = 2 * pc + half
                qs = slice(half * N, (half + 1) * N)
                if routes[jc] == "ts":
                    nc.vector.tensor_scalar(q_t[:, qs], G_b[:, :],
                                            v2_sb[:, jc:jc + 1], v_sb[:, jc:jc + 1],
                                            op0=ALU.mult, op1=ALU.max)
                else:
                    r_t = r32_pool.tile([128, N], F32, tag="rt")
                    nc.scalar.activation(r_t[:, :], S_b[:, :], AF.Relu,
                                         scale=-0.8, bias=negd08[:, jc:jc + 1])
                    nc.scalar.activation(q_t[:, qs], r_t[:, :], AF.Exp,
                                         bias=dT_sb[:, jc:jc + 1])
            p_t = p_pool.tile([128, 2 * N], BF16, tag="pt")
            nc.vector.tensor_tensor(p_t[:, :], q_t[:, :], m_t[:, :], op=ALU.mult)
            for half in (0, 1):
                jc = 2 * pc + half
                for k in range(NSL):
                    nc.tensor.matmul(
                        ps_A[:, k * 512:(k + 1) * 512],
                        V_sb[:, jc * VW:(jc + 1) * VW],
                        p_t[:, half * N + k * 512: half * N + (k + 1) * 512],
                        start=(jc == 0), stop=(jc == NCH - 1))

        # ---------------- tail: normalize ----------------
        tail_pool = ctx.enter_context(tc.tile_pool(name="tail", bufs=1))
        Srow = tail_pool.tile([1, N], F32, tag="Srow")
        for half in range(2):
            hs = slice(half * H2, (half + 1) * H2)
            nc.scalar.copy(Srow[0:1, hs], ps_A[F_OUT:VW, hs])
        nc.scalar.dma_start(sv_dram[:], Srow[:, :])
        sres = tail_pool.tile([128, N // 128], F32, tag="sres")
        nc.scalar.dma_start(sres[:, :], sv_dram.rearrange("(p q) -> p q", p=128))
        rres = tail_pool.tile([128, N // 128], F32, tag="rres")
        nc.vector.reciprocal(rres[:, :], sres[:, :])
        nc.scalar.dma_start(r_dram.rearrange("(p q) -> p q", p=128), rres[:, :])
        R_sb = tail_pool.tile([F_OUT, N], F32, tag="Rsb")
        F_sb = tail_pool.tile([F_OUT, N], F32, tag="Fsb")
        for half in range(2):
            hs = slice(half * H2, (half + 1) * H2)
            nc.sync.dma_start(R_sb[:, hs],
                              r_dram[None, hs].broadcast_to((F_OUT, H2)))
            nc.vector.tensor_tensor(F_sb[:, hs], ps_A[0:F_OUT, hs], R_sb[:, hs],
                                    op=ALU.mult)
            nc.scalar.dma_start(outT[:, hs], F_sb[:, hs])
    nc.compile()
    return nc


_CACHED_NC = None


def _get_nc():
    global _CACHED_NC
    if _CACHED_NC is None:
        _CACHED_NC = build_program()
    return _CACHED_NC


def _split_hilo(x):
    hi = x.astype(ml_dtypes.bfloat16)
    lo = (x - hi.astype(np.float32)).astype(ml_dtypes.bfloat16)
    return hi, lo


def _prep_inputs(h, adj, w, a_src, a_dst, b):
    h = np.asarray(h, dtype=np.float32)
    adj = np.asarray(adj)
    w = np.asarray(w, dtype=np.float32)
    a_src = np.asarray(a_src, dtype=np.float32)
    a_dst = np.asarray(a_dst, dtype=np.float32)
    b = np.asarray(b, dtype=np.float32)

    h_T = np.ascontiguousarray(h.T)
    hT_hi, hT_lo = _split_hilo(h_T)
    # pair-layout mask: maskp[pc, p, half*N + i] = adj[i, (2*pc+half)*128 + p]
    mT = np.ascontiguousarray(adj.T).astype(ml_dtypes.bfloat16)
    maskp = np.ascontiguousarray(
        mT.reshape(NPR, 2, 128, N).transpose(0, 2, 1, 3).reshape(NPR, 128, 2 * N))

    in_maps = []
    for c in range(N_HEAD):
        wa_src = (w[c] @ a_src[c])[:, 0]              # [F_IN]
        wa_dst = (w[c] @ a_dst[c])[:, 0]
        cols = np.stack([wa_src, wa_dst], axis=1)     # [F_IN, 2]
        wa_hi, wa_lo = _split_hilo(cols)
        blob = np.zeros((128, BLOB_W), dtype=ml_dtypes.bfloat16)
        blob[:, 0:2] = wa_hi[0:128]
        blob[:, 2:4] = wa_hi[128:256]
        blob[:, 4:6] = wa_lo[0:128]
        blob[:, 6:8] = wa_lo[128:256]
        blob[:, 8:72] = w[c][0:128].astype(ml_dtypes.bfloat16)
        blob[:, 72:136] = w[c][128:256].astype(ml_dtypes.bfloat16)
        blob[0, 136:136 + F_OUT] = b.astype(ml_dtypes.bfloat16)
        in_maps.append({
            "hT_hi": hT_hi,
            "hT_lo": hT_lo,
            "blob": blob,
            "maskp": maskp,
            "eye2": np.eye(2, dtype=np.float32),
        })
    return in_maps


def _run(in_maps, trace=False, **kwargs):
    nc = _get_nc()
    return run_bass_kernel_spmd(nc, in_maps, list(range(N_HEAD)), trace=trace, **kwargs)


def kernel(h, adj, w, a_src, a_dst, b):
    in_maps = _prep_inputs(h, adj, w, a_src, a_dst, b)
    res = _run(in_maps)
    out = np.stack([np.ascontiguousarray(res.results[c]["outT"].T)
                    for c in range(N_HEAD)])
    return out.astype(np.float32)


# revision 18
# speedup vs baseline: 1.6875x; 1.1649x over previous
"""Trainium2 Bass kernel for nn_MultiHeadGraphAttention (v5).

Multi-head graph attention (GAT-style), one head per NeuronCore:
    h_prime = einsum('nf,hfo->hno', h, w)
    attn    = softmax(where(adj, leakyrelu(s_i + d_j), -inf), axis=-1)
    out     = attn @ h_prime + b

Softmax is invariant to any per-column scale, so with x = s_i + d_j,
v = exp(d), v2 = exp(0.2 d), g = exp(-0.8 s):

    p[i,j] = m * exp(leakyrelu(x)) = exp(s_i) * q[j,i]
    q[j,i] = m * max(v_j, g_i * v2_j)       (exp(s_i) cancels in softmax)

out = (Vaug^T @ q) normalized by the ones-column.  Per-element work per
[128, 4096] chunk of q^T (j on partitions):

  - "ts" route:  ONE fused DVE tensor_scalar  max(G * v2_j, v_j)
  - "act" route: q = exp(0.8*relu(-x) + d_j) via 2 ScalarE passes
  plus one DVE tensor_tensor mask multiply (pair-batched for half the
  pairs, single for the rest -- an A/B experiment).

Preamble: hT loaded as 8 independent half-tiles split across both HWDGE
rings so the s/d matmuls start after ~2MB instead of the full 4MB;
s broadcast across partitions with a PE outer-product (ones x s_row)
into PSUM quarters, from which ScalarE emits G = exp(-0.8 s) (bf16) and
S (f32) directly -- no DRAM bounce.  d transposed into per-partition
columns with PE transposes.  h_prime bias is folded into its matmul via
a ones row; mask pair-DMAs are prefetched before the sd phase.
"""
import sys

if "/opt/trn_rl_repo" not in sys.path:
    sys.path.insert(0, "/opt/trn_rl_repo")

from contextlib import ExitStack

import ml_dtypes
import numpy as np

import concourse.bass as bass
import concourse.bacc as bacc
import concourse.tile as tile
from concourse import mybir
from concourse.bass_utils import run_bass_kernel_spmd

F32 = mybir.dt.float32
BF16 = mybir.dt.bfloat16
AF = mybir.ActivationFunctionType
ALU = mybir.AluOpType

N = 4096
F_IN = 256
N_HEAD = 8
F_OUT = 64
NEG = 0.2
NCH = N // 128        # 32 j-chunks
NPR = NCH // 2        # 16 chunk-pairs
NSL = N // 512        # 8 512-slices
FC = F_IN // 128      # 2 f-chunks
HB = 2048             # half of N (hT half-tile width)
VW = F_OUT + 1        # 65: V columns + ones column

# const blob layout (bf16, [128, BLOB_W]):
#   [0:4) wa2_hi (fc-major)   [4:8) wa2_lo   [8:136) w   [136:200) b row
BLOB_W = 200

# Per-chunk route: "ts" (fused DVE tensor_scalar) or "act" (2x ScalarE).
_ACT = {2, 3, 8, 9, 14, 15, 20, 21, 26, 27, 30}
ROUTES = ["act" if i in _ACT else "ts" for i in range(NCH)]
# pairs with a single fused [128, 8192] mask TT (others: two [128, 4096])
PAIR_TT = set(range(0, NPR, 2))


def build_program(routes=ROUTES):
    nc = bacc.Bacc("TRN2", target_bir_lowering=False, debug=False)
    hT_hi = nc.dram_tensor("hT_hi", [F_IN, N], BF16, kind="ExternalInput").ap()
    hT_lo = nc.dram_tensor("hT_lo", [F_IN, N], BF16, kind="ExternalInput").ap()
    blob = nc.dram_tensor("blob", [128, BLOB_W], BF16, kind="ExternalInput").ap()
    maskp = nc.dram_tensor("maskp", [NPR, 128, 2 * N], BF16,
                           kind="ExternalInput").ap()
    eye2 = nc.dram_tensor("eye2", [2, 2], F32, kind="ExternalInput").ap()
    outT = nc.dram_tensor("outT", [F_OUT, N], F32, kind="ExternalOutput").ap()
    sv_dram = nc.dram_tensor("sv_scratch", [N], F32).ap()
    r_dram = nc.dram_tensor("r_scratch", [N], F32).ap()

    with tile.TileContext(nc) as tc, ExitStack() as ctx:
        const_pool = ctx.enter_context(tc.tile_pool(name="const", bufs=1))
        mask_pool = ctx.enter_context(tc.tile_pool(name="maskpl", bufs=4))
        pre_ctx = ExitStack()
        pre_pool = pre_ctx.enter_context(tc.tile_pool(name="pre", bufs=1))

        # ---- warmup matmuls to release the PE HAM clock gate (no data dep)
        # (own short-lived PSUM pool so it doesn't widen the preamble pools)
        wu_t = pre_pool.tile([128, 512], BF16, tag="wu")
        nc.vector.memset(wu_t[:, :], 0.0)
        with tc.tile_pool(name="pswu", bufs=1, space="PSUM") as pswu_pool:
            ps_wu = pswu_pool.tile([128, 512], F32, tag="pswu")
            for _ in range(8):
                nc.tensor.matmul(ps_wu[:, :], wu_t[:, 0:128], wu_t[:, :],
                                 start=True, stop=True)
        psw_pool = pre_ctx.enter_context(tc.tile_pool(name="psw", bufs=2, space="PSUM"))
        psS_pool = pre_ctx.enter_context(tc.tile_pool(name="psS", bufs=1, space="PSUM"))
        psv_pool = psw_pool

        # ---- input loads: hi halves on sync ring, blob + lo halves on scalar
        # 8 independent half-tiles so the first sd slices start early.
        hThi = [[pre_pool.tile([128, HB], BF16, tag=f"hThi{fc}{ha}",
                               name=f"hThi{fc}{ha}")
                 for ha in range(2)] for fc in range(FC)]
        hTlo = [[pre_pool.tile([128, HB], BF16, tag=f"hTlo{fc}{ha}",
                               name=f"hTlo{fc}{ha}")
                 for ha in range(2)] for fc in range(FC)]
        blob_sb = const_pool.tile([128, BLOB_W], BF16, tag="blob")
        nc.scalar.dma_start(blob_sb[:, :], blob[:, :])
        for ha in range(2):
            for fc in range(FC):
                nc.sync.dma_start(hThi[fc][ha][:, :],
                                  hT_hi[fc * 128:(fc + 1) * 128,
                                        ha * HB:(ha + 1) * HB])
            for fc in range(FC):
                nc.scalar.dma_start(hTlo[fc][ha][:, :],
                                    hT_lo[fc * 128:(fc + 1) * 128,
                                          ha * HB:(ha + 1) * HB])
        eye2_sb = const_pool.tile([2, 2], F32, tag="eye2")
        nc.scalar.dma_start(eye2_sb[:, :], eye2[:, :])
        wa2hi = blob_sb[:, 0:4]
        wa2lo = blob_sb[:, 4:8]
        w_sb = blob_sb[:, 8:136]
        brow_sb = blob_sb[0:1, 136:136 + F_OUT]
        ones_row = const_pool.tile([1, 128], BF16, tag="ones_row")
        nc.vector.memset(ones_row[:, :], 1.0)

        # ---- early mask prefetch (before sd work floods the rings)
        m_tiles = []
        for pc in range(4):
            m_t = mask_pool.tile([128, 2 * N], BF16, tag="mt")
            if pc % 2 == 0:
                nc.sync.dma_start(m_t[:, :], maskp[pc, :, :])
            else:
                nc.scalar.dma_start(m_t[:, :], maskp[pc, :, :])
            m_tiles.append(m_t)

        # ---- s/d rows (hi/lo compensated); s broadcast via PE outer product
        s_bf_row = pre_pool.tile([1, N], BF16, tag="sbfrow")
        sdT = pre_pool.tile([2, N], F32, tag="sdT")
        S_b = const_pool.tile([128, N], F32, tag="Sb")
        G_b = const_pool.tile([128, N], BF16, tag="Gb")
        ps_S = None
        for sl in range(NSL):
            s0 = sl * 512
            ha, off = sl // 4, (sl % 4) * 512
            ps_sd = psw_pool.tile([2, 512], F32, tag="pssd")
            ci = 0
            for fc in range(FC):
                for (wa, ht) in ((wa2hi, hThi[fc][ha]), (wa2hi, hTlo[fc][ha]),
                                 (wa2lo, hThi[fc][ha])):
                    nc.tensor.matmul(ps_sd[:, :], wa[:, fc * 2:(fc + 1) * 2],
                                     ht[:, off:off + 512],
                                     start=(ci == 0), stop=(ci == 5))
                    ci += 1
            nc.scalar.copy(sdT[0:2, s0:s0 + 512], ps_sd[:, :])
            nc.vector.tensor_copy(s_bf_row[0:1, s0:s0 + 512], sdT[0:1, s0:s0 + 512])
            if sl % 2 == 0:
                ps_S = psS_pool.tile([128, 1024], F32, tag="psS")
            nc.tensor.matmul(ps_S[:, (sl % 2) * 512:(sl % 2) * 512 + 512],
                             ones_row[:, :], s_bf_row[0:1, s0:s0 + 512],
                             start=True, stop=True)
            if sl % 2 == 1:
                q0 = (sl - 1) * 512
                nc.scalar.activation(G_b[:, q0:q0 + 1024], ps_S[:, :],
                                     AF.Exp, scale=-0.8)
                nc.scalar.copy(S_b[:, q0:q0 + 1024], ps_S[:, :])

        # ---- d columns via PE transposes -> tables
        ps_dall = psw_pool.tile([128, 2 * NCH], F32, tag="psdall")
        for jc in range(NCH):
            nc.tensor.transpose(ps_dall[:, 2 * jc:2 * jc + 2],
                                sdT[0:2, jc * 128:(jc + 1) * 128], eye2_sb[:, :])
        dT_sb = const_pool.tile([128, NCH], F32, tag="dT")
        nc.vector.tensor_copy(dT_sb[:, :], ps_dall[:, 1::2])
        negd08 = const_pool.tile([128, NCH], F32, tag="negd08")
        nc.vector.tensor_scalar(negd08[:, :], dT_sb[:, :], -0.8, None, op0=ALU.mult)
        v_sb = const_pool.tile([128, NCH], F32, tag="v")
        nc.scalar.activation(v_sb[:, :], dT_sb[:, :], AF.Exp)
        v2_sb = const_pool.tile([128, NCH], F32, tag="v2")
        nc.scalar.activation(v2_sb[:, :], dT_sb[:, :], AF.Exp, scale=NEG)

        # ---- h_prime (V, bf16), bias folded in via ones row; copies on DVE
        V_sb = const_pool.tile([128, NCH * VW], BF16, tag="V")
        nc.vector.memset(V_sb[:, :], 1.0)
        for jc in range(NCH):
            ha, off = jc // 16, (jc % 16) * 128
            ps_v = psv_pool.tile([128, F_OUT], F32, tag="psv")
            for fc in range(FC):
                nc.tensor.matmul(ps_v[:, :], hThi[fc][ha][:, off:off + 128],
                                 w_sb[:, fc * F_OUT:(fc + 1) * F_OUT],
                                 start=(fc == 0), stop=False)
            nc.tensor.matmul(ps_v[:, :], ones_row[:, :], brow_sb[:, :],
                             start=False, stop=True)
            nc.vector.tensor_copy(V_sb[:, jc * VW: jc * VW + F_OUT], ps_v[:, :])

        # ---------------- attention j-loop over chunk PAIRS ----------------
        pre_ctx.close()
        loop_ctx = ExitStack()
        q_pool = loop_ctx.enter_context(tc.tile_pool(name="qp", bufs=2))
        r32_pool = loop_ctx.enter_context(tc.tile_pool(name="r32p", bufs=2))
        p_pool = loop_ctx.enter_context(tc.tile_pool(name="pp", bufs=2))
        psbig_pool = ctx.enter_context(tc.tile_pool(name="psbig", bufs=1, space="PSUM"))
        ps_A = psbig_pool.tile([VW, N], F32, tag="psA")
        for pc in range(NPR):
            if pc < 4:
                m_t = m_tiles[pc]
            else:
                m_t = mask_pool.tile([128, 2 * N], BF16, tag="mt")
                if pc % 2 == 0:
                    nc.sync.dma_start(m_t[:, :], maskp[pc, :, :])
                else:
                    nc.scalar.dma_start(m_t[:, :], maskp[pc, :, :])
            q_t = q_pool.tile([128, 2 * N], BF16, tag="qt")
            for half in (0, 1):
                jc = 2 * pc + half
                qs = slice(half * N, (half + 1) * N)
                if routes[jc] == "ts":
                    nc.vector.tensor_scalar(q_t[:, qs], G_b[:, :],
                                            v2_sb[:, jc:jc + 1], v_sb[:, jc:jc + 1],
                                            op0=ALU.mult, op1=ALU.max)
                else:
                    r_t = r32_pool.tile([128, N], F32, tag="rt")
                    nc.scalar.activation(r_t[:, :], S_b[:, :], AF.Relu,
                                         scale=-0.8, bias=negd08[:, jc:jc + 1])
                    nc.scalar.activation(q_t[:, qs], r_t[:, :], AF.Exp,
                                         bias=dT_sb[:, jc:jc + 1])
            p_t = p_pool.tile([128, 2 * N], BF16, tag="pt")
            if pc in PAIR_TT:
                nc.vector.tensor_tensor(p_t[:, :], q_t[:, :], m_t[:, :],
                                        op=ALU.mult)
            else:
                for half in (0, 1):
                    qs = slice(half * N, (half + 1) * N)
                    nc.vector.tensor_tensor(p_t[:, qs], q_t[:, qs], m_t[:, qs],
                                            op=ALU.mult)
            for half in (0, 1):
                jc = 2 * pc + half
                for k in range(NSL):
                    nc.tensor.matmul(
                        ps_A[:, k * 512:(k + 1) * 512],
                        V_sb[:, jc * VW:(jc + 1) * VW],
                        p_t[:, half * N + k * 512: half * N + (k + 1) * 512],
                        start=(jc == 0), stop=(jc == NCH - 1))

        # ---------------- tail: normalize ----------------
        loop_ctx.close()
        H2 = N // 2
        tail_pool = ctx.enter_context(tc.tile_pool(name="tail", bufs=1))
        Srow = tail_pool.tile([1, N], F32, tag="Srow")
        for half in range(2):
            hs = slice(half * H2, (half + 1) * H2)
            nc.scalar.copy(Srow[0:1, hs], ps_A[F_OUT:VW, hs])
        nc.scalar.dma_start(sv_dram[:], Srow[:, :])
        sres = tail_pool.tile([128, N // 128], F32, tag="sres")
        nc.scalar.dma_start(sres[:, :], sv_dram.rearrange("(p q) -> p q", p=128))
        rres = tail_pool.tile([128, N // 128], F32, tag="rres")
        nc.vector.reciprocal(rres[:, :], sres[:, :])
        nc.scalar.dma_start(r_dram.rearrange("(p q) -> p q", p=128), rres[:, :])
        R_sb = tail_pool.tile([F_OUT, N], F32, tag="Rsb")
        F_sb = tail_pool.tile([F_OUT, N], F32, tag="Fsb")
        for half in range(2):
            hs = slice(half * H2, (half + 1) * H2)
            nc.sync.dma_start(R_sb[:, hs],
                              r_dram[None, hs].broadcast_to((F_OUT, H2)))
            nc.vector.tensor_tensor(F_sb[:, hs], ps_A[0:F_OUT, hs], R_sb[:, hs],
                                    op=ALU.mult)
            nc.scalar.dma_start(outT[:, hs], F_sb[:, hs])
    nc.compile()
    return nc


_CACHED_NC = None


def _get_nc():
    global _CACHED_NC
    if _CACHED_NC is None:
        _CACHED_NC = build_program()
    return _CACHED_NC


def _split_hilo(x):
    hi = x.astype(ml_dtypes.bfloat16)
    lo = (x - hi.astype(np.float32)).astype(ml_dtypes.bfloat16)
    return hi, lo


def _prep_inputs(h, adj, w, a_src, a_dst, b):
    h = np.asarray(h, dtype=np.float32)
    adj = np.asarray(adj)
    w = np.asarray(w, dtype=np.float32)
    a_src = np.asarray(a_src, dtype=np.float32)
    a_dst = np.asarray(a_dst, dtype=np.float32)
    b = np.asarray(b, dtype=np.float32)

    h_T = np.ascontiguousarray(h.T)
    hT_hi, hT_lo = _split_hilo(h_T)
    # pair-layout mask: maskp[pc, p, half*N + i] = adj[i, (2*pc+half)*128 + p]
    mT = np.ascontiguousarray(adj.T).astype(ml_dtypes.bfloat16)
    maskp = np.ascontiguousarray(
        mT.reshape(NPR, 2, 128, N).transpose(0, 2, 1, 3).reshape(NPR, 128, 2 * N))

    in_maps = []
    for c in range(N_HEAD):
        wa_src = (w[c] @ a_src[c])[:, 0]              # [F_IN]
        wa_dst = (w[c] @ a_dst[c])[:, 0]
        cols = np.stack([wa_src, wa_dst], axis=1)     # [F_IN, 2]
        wa_hi, wa_lo = _split_hilo(cols)
        blob = np.zeros((128, BLOB_W), dtype=ml_dtypes.bfloat16)
        blob[:, 0:2] = wa_hi[0:128]
        blob[:, 2:4] = wa_hi[128:256]
        blob[:, 4:6] = wa_lo[0:128]
        blob[:, 6:8] = wa_lo[128:256]
        blob[:, 8:72] = w[c][0:128].astype(ml_dtypes.bfloat16)
        blob[:, 72:136] = w[c][128:256].astype(ml_dtypes.bfloat16)
        blob[0, 136:136 + F_OUT] = b.astype(ml_dtypes.bfloat16)
        in_maps.append({
            "hT_hi": hT_hi,
            "hT_lo": hT_lo,
            "blob": blob,
            "maskp": maskp,
            "eye2": np.eye(2, dtype=np.float32),
        })
    return in_maps


def _run(in_maps, trace=False, **kwargs):
    nc = _get_nc()
    return run_bass_kernel_spmd(nc, in_maps, list(range(N_HEAD)), trace=trace, **kwargs)


def kernel(h, adj, w, a_src, a_dst, b):
    in_maps = _prep_inputs(h, adj, w, a_src, a_dst, b)
    res = _run(in_maps)
    out = np.stack([np.ascontiguousarray(res.results[c]["outT"].T)
                    for c in range(N_HEAD)])
    return out.astype(np.float32)
